# revision 1
# baseline (speedup 1.0000x reference)
"""GATv2 3-layer GNN on 8 TRN2 NeuronCores.

Sharding: edges sorted by destination node; dst-range sharded across the 8
cores (1250 nodes per core).  Per layer each core computes xl/xr for its own
node slice, the xl slices are all-gathered (bf16) so every core can gather
xl[src] for arbitrary src, and each core aggregates messages for its dst
range only (no cross-core reduction needed).

Edge phase per core: edges are grouped per 128-node dst block and processed
in 128-edge groups.  For each group:
  - one-hot matrix oh[e, n] = (dst_local[e] == n) built by DVE is_equal
  - m = oh.T-gather of xr_block + edge_attr @ We + gathered xl[src]
    (three PE matmuls accumulating in PSUM)
  - logits  e = sum_c lrelu(m) * att   (ACT lrelu, DVE mul + segmented reduce)
  - ex = exp(e)  (no segment-max subtraction: alpha = ex/sum(ex) is exact
    up to fp rounding and the logits are tame)
  - scatter-add of [ex * xl_src | ex] into the block accumulator via a PE
    matmul with lhsT = oh  (PSUM accumulation over the block's groups)
Block epilogue: out = acc / (den + eps), inter-layer leaky_relu fused into
the PE-transpose evacuation that produces the next layer's hT.
"""

import os
import numpy as np
import ml_dtypes

# ---- problem constants (hardcoded per spec nn_GATv2_5454608466160) ----
N = 10000
E = 160000
F_NODE = 16
F_EDGE = 8
H = 8
C = 64
HC = 512
NCLS = 4
SLOPE = 0.2
EPS = 1e-16

NCORES = 8
SLICE = 1250          # real nodes owned per core
BLK = 119             # dst nodes per block (119 + 8 edge-attr rows + 1 = 128 = K)
NBLK = 11             # blocks per core (11*119 = 1309 >= 1250)
SLICE_PAD = BLK * NBLK
NPAD = NCORES * SLICE_PAD  # rows in the gathered xl table

BF16 = ml_dtypes.bfloat16

_cache = {}


# --------------------------------------------------------------------------
# host-side preprocessing
# --------------------------------------------------------------------------
def _prep_edges(edge_index, edge_attr):
    """Sort edges by dst, shard by dst range, block by 128 dst nodes,
    pad each (core, block) to a common group count."""
    src = edge_index[0].astype(np.int64)
    dst = edge_index[1].astype(np.int64)
    order = np.argsort(dst, kind="stable")
    src, dst = src[order], dst[order]
    ea = edge_attr[order]

    core_of = dst // SLICE
    # edge counts per (core, block)
    blk_of = (dst - core_of * SLICE) // BLK
    counts = np.zeros((NCORES, NBLK), dtype=np.int64)
    for k in range(NCORES):
        m = core_of == k
        counts[k] = np.bincount(blk_of[m], minlength=NBLK)
    G = np.maximum(1, np.ceil(counts.max(axis=0) / 128).astype(np.int64))  # groups per block
    e_pad = int(G.sum() * 128)

    src_pos = np.zeros((NCORES, e_pad), dtype=np.int64)
    dst_loc = np.full((NCORES, e_pad), -1.0, dtype=np.float32)
    ea_pad = np.zeros((NCORES, e_pad, F_EDGE), dtype=np.float32)
    boff = np.concatenate([[0], np.cumsum(G * 128)])  # edge offset of each block

    for k in range(NCORES):
        mk = core_of == k
        sk, dk, eak, bk = src[mk], dst[mk], ea[mk], blk_of[mk]
        for b in range(NBLK):
            mb = bk == b
            n = int(mb.sum())
            o = int(boff[b])
            src_pos[k, o:o + n] = sk[mb]
            dst_loc[k, o:o + n] = (dk[mb] - k * SLICE - b * BLK).astype(np.float32)
            ea_pad[k, o:o + n] = eak[mb]

    # src -> row position in the all-gathered xl table
    s_slice = src_pos // SLICE
    s_local = src_pos - s_slice * SLICE
    gpos = (s_slice * SLICE_PAD + s_local).astype(np.int16)

    # dma_gather index wrap: idx i at [i % 16, i // 16], tiled to 128 partitions
    gidx = np.ascontiguousarray(
        gpos.reshape(NCORES, e_pad // 16, 16).transpose(0, 2, 1))
    gidx = np.tile(gidx, (1, 8, 1))                      # [NCORES, 128, e_pad//16]

    # dst_local laid out [128, G_total]: group g's edge j at [j, g]
    g_tot = e_pad // 128
    dl = np.ascontiguousarray(
        dst_loc.reshape(NCORES, g_tot, 128).transpose(0, 2, 1)).astype(np.float32)

    # edge_attr transposed, padded with a zero row -> [F_EDGE+1, e_pad]
    eaT = np.ascontiguousarray(ea_pad.transpose(0, 2, 1)).astype(BF16)
    eaT = np.concatenate(
        [eaT, np.zeros((NCORES, 1, e_pad), dtype=BF16)], axis=1)

    return {
        "G": tuple(int(g) for g in G),
        "e_pad": e_pad,
        "gidx": gidx,
        "dst_loc": dl,
        "dst_locF": dst_loc.reshape(NCORES, 1, e_pad).astype(BF16),
        "eaT": eaT,
    }


def _to_bf16(x):
    return np.asarray(x, dtype=np.float32).astype(BF16)


# --------------------------------------------------------------------------
# device kernel build
# --------------------------------------------------------------------------
def _build(G, e_pad, nonzero_bias, trunc=None):
    import concourse.bass as bass
    import concourse.bacc as bacc
    import concourse.mybir as mybir
    import concourse.tile as tile
    from concourse import library_config

    f32 = mybir.dt.float32
    bf16 = mybir.dt.bfloat16
    i16 = mybir.dt.int16
    AF = mybir.ActivationFunctionType
    OP = mybir.AluOpType

    g_tot = e_pad // 128
    boff = [0]
    for g in G:
        boff.append(boff[-1] + g)

    import os as _os
    trunc = trunc or _os.environ.get("KTRUNC", "")
    nc = bacc.Bacc("TRN2", target_bir_lowering=False, debug=False,
                   num_devices=NCORES)

    # ---- I/O ----
    def inp(name, shape, dt=bf16):
        return nc.dram_tensor(name, shape, dt, kind="ExternalInput")

    xT0 = inp("xT0", [F_NODE, SLICE_PAD])
    Wls = [inp(f"Wl{l}", [F_NODE if l == 0 else HC, HC]) for l in range(3)]
    Wrs = [inp(f"Wr{l}", [F_NODE if l == 0 else HC, HC]) for l in range(3)]
    Wes = [inp(f"We{l}", [F_EDGE, HC]) for l in range(3)]
    attBs = [inp(f"attB{l}", [128, HC]) for l in range(3)]
    Wf = inp("Wf", [HC, NCLS])
    eaT_h = inp("eaT", [F_EDGE + 1, e_pad])
    dl_h = inp("dstloc", [128, g_tot], f32)
    dlF_h = inp("dstlocF", [1, e_pad])
    icol_h = inp("iotacol", [128, 1], f32)
    ones1_h = inp("ones1", [1, 128])
    gidx_h = inp("gidx", [128, e_pad // 16], i16)
    iota_h = inp("iota", [128, 128])
    I128_h = inp("ident", [128, 128])
    biasB = None
    if nonzero_bias:
        # per-layer broadcast biases: bl+br is folded into xl evac; bo into epilogue
        biasB = {
            "lr": [inp(f"blrB{l}", [128, HC]) for l in range(3)],   # bl (for xl)
            "rr": [inp(f"brrB{l}", [128, HC]) for l in range(3)],   # br (for xr)
            "bo": [inp(f"boB{l}", [128, HC]) for l in range(3)],
            "bf": inp("bfB", [128, NCLS]),
        }

    out_h = nc.dram_tensor("out", [SLICE_PAD, NCLS], f32, kind="ExternalOutput")

    with tile.TileContext(nc) as tc:
        import contextlib
        ctx = contextlib.ExitStack()
        with ctx:
            cpool = ctx.enter_context(tc.tile_pool(name="const", bufs=1))
            wpool = ctx.enter_context(tc.tile_pool(name="weights", bufs=1))
            hpool = ctx.enter_context(tc.tile_pool(name="hT", bufs=1))
            spool = ctx.enter_context(tc.tile_pool(name="slices", bufs=1))
            gpool = ctx.enter_context(tc.tile_pool(name="gather", bufs=4))
            bpool = ctx.enter_context(tc.tile_pool(name="blk", bufs=2))
            epool = ctx.enter_context(tc.tile_pool(name="edge", bufs=10))
            mpool = ctx.enter_context(tc.tile_pool(name="mact", bufs=6))
            dpool = ctx.enter_context(tc.tile_pool(name="dram", bufs=1, space="DRAM"))
            ps_m = ctx.enter_context(tc.tile_pool(name="ps_m", bufs=3, space="PSUM"))
            ps_b = ctx.enter_context(tc.tile_pool(name="ps_b", bufs=1, space="PSUM"))
            ps_o = ctx.enter_context(tc.tile_pool(name="ps_o", bufs=2, space="PSUM"))
            ps_d = ctx.enter_context(tc.tile_pool(name="ps_d", bufs=1, space="PSUM"))
            ps_t = ctx.enter_context(tc.tile_pool(name="ps_t", bufs=1, space="PSUM"))

            nc.gpsimd.load_library(library_config.mlp)

            # ---- load constants ----
            _ldn = [0]
            def load(pool, src_ap, shape, dt=bf16, tag=None):
                _ldn[0] += 1
                t = pool.tile(shape, dt, tag=tag or f"cst{_ldn[0]}")
                nc.sync.dma_start(t[:], src_ap)
                return t

            iota_sb = load(cpool, iota_h[:, :], [128, 128])
            I128_sb = load(cpool, I128_h[:, :], [128, 128])
            attB_sb = [load(cpool, attBs[l][:, :], [128, HC]) for l in range(3)]
            We_sb = [load(cpool, Wes[l][:, :], [F_EDGE, HC]) for l in range(3)]
            dl_sb = load(cpool, dl_h[:, :], [128, g_tot], f32)
            dlF_sb = load(cpool, dlF_h[:, :], [1, e_pad])
            icol_sb = load(cpool, icol_h[:, :], [128, 1], f32)
            ones1_sb = load(cpool, ones1_h[:, :], [1, 128])
            gidx_sb = load(cpool, gidx_h[:, :], [128, e_pad // 16], i16)
            # Wf as [128, 4, NCLS] (row k -> [k%128, k//128])
            Wf_sb = load(cpool, Wf.ap().rearrange("(kc p) n -> p kc n", p=128),
                         [128, 4, NCLS])
            bias_sb = None
            if nonzero_bias:
                bias_sb = {
                    "lr": [load(cpool, biasB["lr"][l][:, :], [128, HC]) for l in range(3)],
                    "rr": [load(cpool, biasB["rr"][l][:, :], [128, HC]) for l in range(3)],
                    "bo": [load(cpool, biasB["bo"][l][:, :], [128, HC]) for l in range(3)],
                    "bf": load(cpool, biasB["bf"][:, :], [128, NCLS]),
                }

            # layer weights: layer 0 [16, HC]; layers 1,2 [128, 4, HC]
            Wl_sb, Wr_sb = [], []
            for l in range(3):
                if l == 0:
                    Wl_sb.append(load(wpool, Wls[0][:, :], [F_NODE, HC]))
                    Wr_sb.append(load(wpool, Wrs[0][:, :], [F_NODE, HC]))
                else:
                    Wl_sb.append(load(
                        wpool, Wls[l].ap().rearrange("(kc p) n -> p kc n", p=128),
                        [128, 4, HC]))
                    Wr_sb.append(load(
                        wpool, Wrs[l].ap().rearrange("(kc p) n -> p kc n", p=128),
                        [128, 4, HC]))

            xT0_sb = load(hpool, xT0[:, :], [F_NODE, SLICE_PAD])

            hT = None  # [128, 4, SLICE_PAD] bf16 for layers >= 1

            for l in range(3):
                KCH = 1 if l == 0 else 4

                # ---------------- node phase ----------------
                xl_sl = spool.tile([128, NBLK, HC], bf16, tag="xl_sl")
                xr_sl = spool.tile([128, NBLK, HC], bf16, tag="xr_sl")
                for cblk in range(NBLK):
                    for which, Wsb, dst_t in (("l", Wl_sb[l], xl_sl),
                                              ("r", Wr_sb[l], xr_sl)):
                        ps = ps_m.tile([128, HC], f32, tag="m_ps")
                        for kc in range(KCH):
                            if l == 0:
                                lhsT = xT0_sb[:, cblk * BLK:(cblk + 1) * BLK]
                                rhs = Wsb[:, :]
                            else:
                                lhsT = hT[:, kc, cblk * BLK:(cblk + 1) * BLK]
                                rhs = Wsb[:, kc, :]
                            nc.tensor.matmul(ps[0:BLK, :], lhsT, rhs,
                                             start=(kc == 0), stop=(kc == KCH - 1))
                        if nonzero_bias:
                            b = bias_sb["lr" if which == "l" else "rr"][l]
                            nc.vector.tensor_tensor(
                                out=dst_t[0:BLK, cblk, :], in0=ps[0:BLK, :],
                                in1=b[0:BLK, :], op=OP.add)
                        else:
                            nc.scalar.activation(dst_t[0:BLK, cblk, :],
                                                 ps[0:BLK, :], AF.Copy)

                if trunc == "node" and l == 0:
                    nc.gpsimd.dma_start(out_h.ap().rearrange("(s p) n -> p s n", p=128),
                                      xl_sl[:, 0:NBLK, 0:NCLS])
                    break
                # ---------------- all-gather xl ----------------
                xl_dram = dpool.tile([SLICE_PAD, HC], bf16, tag="xl_dram")
                xl_full = dpool.tile([NPAD, HC], bf16, tag="xl_full")
                # SBUF [128, NBLK, HC] -> DRAM rows j = s*128+p
                nc.sync.dma_start(
                    xl_dram[:].rearrange("(s p) n -> p s n", p=BLK),
                    xl_sl[0:BLK, :, :])
                nc.gpsimd.collective_compute(
                    "AllGather",
                    mybir.AluOpType.bypass,
                    replica_groups=[list(range(NCORES))],
                    ins=[xl_dram[:].opt()],
                    outs=[xl_full[:].opt()],
                )

                if trunc == "ag" and l == 0:
                    tmp = spool.tile([128, NBLK, NCLS], f32, tag="tmpo")
                    nc.gpsimd.dma_start(tmp[:], xl_full[0:1280, 0:NCLS].rearrange("(s p) n -> p s n", p=128))
                    nc.sync.dma_start(out_h.ap().rearrange("(s p) n -> p s n", p=128), tmp[:])
                    break
                # ---------------- edge phase ----------------
                hT_next = hpool.tile([128, 4, SLICE_PAD], bf16, tag="hT_next")
                nblk_eff = 1 if (trunc == "1blk" and l == 0) else NBLK
                for b in range(nblk_eff):
                    Gb = G[b]
                    rhs_blk = bpool.tile([128, HC], bf16, tag="rhsb")
                    nc.vector.memset(rhs_blk[:], 0.0)
                    nc.scalar.activation(rhs_blk[0:BLK, :], xr_sl[0:BLK, b, :],
                                         AF.Copy)
                    nc.sync.dma_start(rhs_blk[BLK:BLK + F_EDGE, :], We_sb[l][:])
                    acc = ps_o.tile([128, HC], f32, tag="acc")
                    den = ps_d.tile([128, F_EDGE], f32, tag="den")
                    chunks = [(g0, min(8, Gb - g0)) for g0 in range(0, Gb, 8)]
                    xgs, eas = {}, {}
                    for (g0, gn) in chunks:
                        xg = gpool.tile([128, 8, HC], bf16, tag="xg")
                        nc.gpsimd.dma_gather(
                            xg[:, 0:gn, :],
                            xl_full[:],
                            gidx_sb[:, (boff[b] + g0) * 8:(boff[b] + g0 + gn) * 8],
                            gn * 128,
                            gn * 128,
                            HC,
                        )
                        ea_sb = gpool.tile([F_EDGE + 1, 8 * 128], bf16, tag="ea")
                        nc.sync.dma_start(
                            ea_sb[:, 0:gn * 128],
                            eaT_h[:, (boff[b] + g0) * 128:(boff[b] + g0 + gn) * 128])
                        xgs[g0], eas[g0] = xg, ea_sb
                    ohT4s = {}
                    for q0 in range(0, Gb, 4):
                        qn = min(4, Gb - q0)
                        dB_ps = ps_b.tile([128, 512], f32, tag="dbp")
                        nc.tensor.matmul(
                            dB_ps[:, 0:qn * 128], ones1_sb[:],
                            dlF_sb[:, (boff[b] + q0) * 128:(boff[b] + q0 + qn) * 128],
                            start=True, stop=True)
                        dB = epool.tile([128, 512], bf16, tag="dB")
                        nc.scalar.activation(dB[:, 0:qn * 128], dB_ps[:, 0:qn * 128],
                                             AF.Copy)
                        ohT4 = epool.tile([128, 512], bf16, tag="ohT4")
                        nc.vector.tensor_scalar(
                            out=ohT4[:, 0:qn * 128], in0=dB[:, 0:qn * 128],
                            scalar1=icol_sb[:, 0:1], scalar2=None,
                            op0=OP.is_equal)
                        ohT4s[q0] = ohT4
                    for g in range(Gb):
                        col = boff[b] + g
                        xg, ea_sb = xgs[g - g % 8], eas[g - g % 8]
                        ohT = ohT4s[g - g % 4][:, (g % 4) * 128:(g % 4 + 1) * 128]
                        oh_ps = ps_t.tile([128, 128], bf16, tag="trp")
                        nc.tensor.transpose(oh_ps[:], ohT, I128_sb[:])
                        oh = epool.tile([128, 128], bf16, tag="oh")
                        nc.scalar.activation(oh[:], oh_ps[:], AF.Copy)
                        comb = epool.tile([128, 128], bf16, tag="comb")
                        nc.scalar.activation(comb[0:BLK, :], ohT[0:BLK, :], AF.Copy)
                        nc.sync.dma_start(
                            comb[BLK:128, :],
                            ea_sb[:, (g % 8) * 128:(g % 8 + 1) * 128])

                        m_ps = ps_m.tile([128, HC], f32, tag="m_ps")
                        nc.tensor.matmul(m_ps[:], comb[:], rhs_blk[:],
                                         start=True, stop=False)
                        nc.tensor.matmul(m_ps[:], I128_sb[:], xg[:, g % 8, :],
                                         start=False, stop=True)

                        ma = mpool.tile([128, HC], bf16, tag="ma")
                        nc.scalar.activation(ma[:], m_ps[:], AF.Prelu, alpha=SLOPE)
                        m2 = mpool.tile([128, HC], bf16, tag="m2")
                        nc.vector.tensor_tensor(out=m2[:], in0=ma[:],
                                                in1=attB_sb[l][:], op=OP.mult)
                        elog = epool.tile([128, H], f32, tag="elog")
                        nc.vector.tensor_reduce(
                            elog[:], m2[:].rearrange("p (h c) -> p h c", c=C),
                            axis=mybir.AxisListType.X, op=OP.add)
                        ex = epool.tile([128, H], bf16, tag="ex")
                        nc.scalar.activation(ex[:], elog[:], AF.Exp)

                        w = mpool.tile([128, HC], bf16, tag="w")
                        nc.vector.tensor_tensor(
                            out=w[:].rearrange("p (h c) -> p h c", c=C),
                            in0=xg[:, g % 8, :].rearrange("p (h c) -> p h c", c=C),
                            in1=ex[:].unsqueeze(2).broadcast_to([128, H, C]),
                            op=OP.mult)

                        nc.tensor.matmul(acc[:], oh[:], w[:],
                                         start=(g == 0), stop=(g == Gb - 1))
                        nc.tensor.matmul(den[:], oh[:], ex[:],
                                         start=(g == 0), stop=(g == Gb - 1))

                    # ---- block epilogue ----
                    dinv = epool.tile([128, H], f32, tag="dinv")
                    nc.vector.tensor_scalar(out=dinv[:], in0=den[:],
                                            scalar1=EPS, scalar2=None,
                                            op0=OP.add)
                    nc.vector.reciprocal(dinv[:], dinv[:])
                    hpre = mpool.tile([128, HC], bf16, tag="hpre")
                    nc.vector.tensor_tensor(
                        out=hpre[:].rearrange("p (h c) -> p h c", c=C),
                        in0=acc[:].rearrange("p (h c) -> p h c", c=C),
                        in1=dinv[:].unsqueeze(2).broadcast_to([128, H, C]),
                        op=OP.mult)
                    if nonzero_bias:
                        nc.vector.tensor_tensor(out=hpre[:], in0=hpre[:],
                                                in1=bias_sb["bo"][l][:], op=OP.add)
                    if trunc in ("1blk", "1layer") and l == 0 and b == nblk_eff - 1:
                        nc.gpsimd.dma_start(
                            out_h.ap().rearrange("(s p) n -> p s n", p=128),
                            hpre[:, 0:NBLK * NCLS].rearrange("p (s n) -> p s n", n=NCLS))
                        break
                    if l < 2:
                        for fc in range(4):
                            tr = ps_t.tile([128, 128], bf16, tag="trp")
                            nc.tensor.transpose(
                                tr[:], hpre[:, fc * 128:(fc + 1) * 128], I128_sb[:])
                            nc.scalar.activation(
                                hT_next[:, fc, b * BLK:(b + 1) * BLK],
                                tr[:, 0:BLK], AF.Prelu, alpha=SLOPE)
                    else:
                        # final layer: classifier on lrelu(h3)
                        h3 = mpool.tile([128, HC], bf16, tag="h3")
                        nc.scalar.activation(h3[:], hpre[:], AF.Prelu, alpha=SLOPE)
                        # out[128 nodes, NCLS] = sum_k h3T_chunk.T @ Wf_chunk:
                        # need h3 transposed; use PE transposes
                        ops = ps_m.tile([128, NCLS], f32, tag="m_ps")
                        for fc in range(4):
                            tr = ps_t.tile([128, 128], bf16, tag="trp")
                            nc.tensor.transpose(
                                tr[:], h3[:, fc * 128:(fc + 1) * 128], I128_sb[:])
                            h3T = epool.tile([128, 128], bf16, tag="h3T")
                            nc.scalar.activation(h3T[:], tr[:], AF.Copy)
                            nc.tensor.matmul(ops[:], h3T[:], Wf_sb[:, fc, :],
                                             start=(fc == 0), stop=(fc == 3))
                        if nonzero_bias:
                            osb = mpool.tile([128, NCLS], f32, tag="osb")
                            nc.vector.tensor_tensor(out=osb[:], in0=ops[:],
                                                    in1=bias_sb["bf"][:], op=OP.add)
                        else:
                            osb = mpool.tile([128, NCLS], f32, tag="osb")
                            nc.scalar.activation(osb[:], ops[:], AF.Copy)
                        nc.sync.dma_start(
                            out_h.ap().rearrange("(s p) n -> p s n", p=BLK)
                            [:, b, :], osb[0:BLK, :])
                if trunc in ("node", "ag", "1blk", "1layer") and l == 0:
                    break
                if l < 2:
                    hT = hT_next

    nc.compile()
    return nc


# --------------------------------------------------------------------------
# public entry point
# --------------------------------------------------------------------------
def _get_compiled(inputs):
    prep = _prep_edges(inputs["edge_index"], inputs["edge_attr"])
    nonzero_bias = any(
        np.abs(inputs[k]).max() > 0
        for k in ("bl0", "br0", "bo0", "bl1", "br1", "bo1",
                  "bl2", "br2", "bo2", "bf"))
    key = (prep["G"], prep["e_pad"], nonzero_bias)
    if key not in _cache:
        _cache[key] = _build(prep["G"], prep["e_pad"], nonzero_bias)
    return _cache[key], prep, nonzero_bias


def _make_in_maps(inputs, prep, nonzero_bias):
    x = np.asarray(inputs["x"], dtype=np.float32)
    xpad = np.zeros((NCORES, SLICE_PAD, F_NODE), dtype=np.float32)
    xr = x.reshape(NCORES, SLICE, F_NODE)
    xpad[:, :SLICE] = xr
    iota = np.tile(np.arange(128, dtype=np.float32), (128, 1)).astype(BF16)
    iotacol = np.arange(128, dtype=np.float32).reshape(128, 1)
    ones1 = np.ones((1, 128), dtype=np.float32).astype(BF16)
    ident = np.eye(128, dtype=np.float32).astype(BF16)

    common = {
        "iota": iota,
        "iotacol": iotacol,
        "ones1": ones1,
        "ident": ident,
        "Wf": _to_bf16(inputs["Wf"]),
    }
    for l in range(3):
        common[f"Wl{l}"] = _to_bf16(inputs[f"Wl{l}"])
        common[f"Wr{l}"] = _to_bf16(inputs[f"Wr{l}"])
        common[f"We{l}"] = _to_bf16(inputs[f"We{l}"])
        att = np.asarray(inputs[f"att{l}"], np.float32).reshape(1, HC)
        common[f"attB{l}"] = _to_bf16(np.tile(att, (128, 1)))
    if nonzero_bias:
        for l in range(3):
            common[f"blrB{l}"] = _to_bf16(
                np.tile(np.asarray(inputs[f"bl{l}"]).reshape(1, HC), (128, 1)))
            common[f"brrB{l}"] = _to_bf16(
                np.tile(np.asarray(inputs[f"br{l}"]).reshape(1, HC), (128, 1)))
            common[f"boB{l}"] = _to_bf16(
                np.tile(np.asarray(inputs[f"bo{l}"]).reshape(1, HC), (128, 1)))
        common["bfB"] = _to_bf16(
            np.tile(np.asarray(inputs["bf"]).reshape(1, NCLS), (128, 1)))

    in_maps = []
    for k in range(NCORES):
        m = dict(common)
        m["xT0"] = np.ascontiguousarray(xpad[k].T).astype(BF16)
        m["eaT"] = prep["eaT"][k]
        m["dstloc"] = prep["dst_loc"][k]
        m["dstlocF"] = prep["dst_locF"][k]
        m["gidx"] = prep["gidx"][k]
        in_maps.append(m)
    return in_maps


def run(inputs, trace=False, **kw):
    from concourse.bass_utils import run_bass_kernel_spmd
    nc, prep, nonzero_bias = _get_compiled(inputs)
    in_maps = _make_in_maps(inputs, prep, nonzero_bias)
    res = run_bass_kernel_spmd(nc, in_maps, core_ids=list(range(NCORES)),
                               trace=trace, **kw)
    outs = [res.results[k]["out"][:SLICE] for k in range(NCORES)]
    full = np.concatenate(outs, axis=0).astype(np.float32)
    return full, res


def kernel(**inputs):
    out, _ = run(inputs, trace=False)
    return out

